# revision 45
# baseline (speedup 1.0000x reference)
"""Trainium2 Bass kernel for nn_ClusteringLayer: per-cluster nearest-token retrieval.

reference: d2[t,k] = ||x_t||^2 + ||c_k||^2 - 2 x_t.c_k ; indices[k] = argmin_t d2;
output = x[indices]  (shape (1, 64, 128), fp32).

Strategy (8-way token-parallel, memory-regime):
  * Device pass is a SCREEN, not the final answer. The host sorts tokens by
    ||x||^2, casts to fp8e4m3, pre-transposes to feature-major [128, n], and
    shards the sorted order across the 8 cores. Each core computes only the
    cross term xc[k,t] = (2c_k).x_t (one K=128 fp8 matmul per 512-token
    segment; two segments pack into each [128, 512] PSUM tile via
    column-group tile_position) and reduces each segment to one value per
    cluster. Because tokens are x2-sorted, ||x||^2 is nearly constant within
    a segment, so max_t(2xc - x2) is bracketed by the device's max_t(2xc)
    minus the segment's known [x2min, x2max] — no x2 work on device at all.
  * Per-segment-pair reduction alternates engines: DVE reduce-max for
    gp%2==0, ScalarE exp-sum for the rest (activation(Exp, accum_out): a
    log-sum-exp upper bound with slack <= ln(512)/BETA), splitting the
    PSUM-read wall across both otherwise-idle engines.
  * The host turns the per-(cluster, segment) screen values into upper/lower
    brackets of the true per-segment max of S = 2xc - x2, keeps every
    segment whose upper bracket clears the best lower bracket (sound under
    fp8 error EPS8, LSE slack, and x2 spread), rescores those few segments
    exactly in fp32 with the reference formula, and gathers winners from the
    original fp32 x — the final output is exact.
"""

import numpy as np
import ml_dtypes

BF = ml_dtypes.bfloat16

N_TOKENS = 1_000_000
D = 128
K = 64
N_CORES = 8
SEG = 512                 # tokens per matmul / per reduced segment
PAIR = 2 * SEG            # tokens per PSUM tile (two column-group matmuls)
TOK_PER_CORE = N_TOKENS // N_CORES          # 125000
NPAIR = -(-TOK_PER_CORE // PAIR)            # 123
PTOK = NPAIR * PAIR                         # 125952 (952 pad tokens / core)
CHUNK = 4096              # tokens per DMA chunk (multiple of PAIR)
BETA = 2.0                # exp-screen sharpness; LSE slack = ln(512)/BETA ~ 3.1
# reduction-engine schedule, one entry per gp%5: 0 -> DVE pair, 2/3 -> first/
# second half of a 2-pair ScalarE exp-sum group (3 DVE : 2 ACT measured best)
PAIR_KIND = (0, 0, 0, 2, 3)
EPS8 = 12.0               # 2x max observed fp8 screen error (~5.6)
PAD_NEG = -1.0e9


def _np_f8():
    from concourse import mybir

    return mybir.dt.np(mybir.dt.float8e4)


def _build_nc(ptok, chunk):
    from contextlib import ExitStack

    import concourse.bacc as bacc
    import concourse.tile as tile
    from concourse import mybir

    f32 = mybir.dt.float32
    f8 = mybir.dt.float8e4

    npair = ptok // PAIR
    nc = bacc.Bacc()
    xt = nc.declare_dram_parameter("xt", [128, ptok], f8, isOutput=False)
    wc = nc.declare_dram_parameter("wc", [128, K], f8, isOutput=False)
    cb = nc.declare_dram_parameter("cb", [128, 1], f32, isOutput=False)
    tm = nc.declare_dram_parameter("tm", [128, npair], f32, isOutput=True)

    # ramp-up chunk schedule: small first chunks so the first matmuls start
    # as soon as possible instead of waiting on a full-size DMA
    sizes = []
    rem = ptok
    for s in (1024, 2048, 4096):
        if rem >= s:
            sizes.append(s)
            rem -= s
    while rem > 0:
        s = min(chunk, rem)
        sizes.append(s)
        rem -= s

    with tile.TileContext(nc) as tc, ExitStack() as ctx:
        const = ctx.enter_context(tc.tile_pool(name="const", bufs=1))
        xpool = ctx.enter_context(tc.tile_pool(name="xpool", bufs=6))
        spool = ctx.enter_context(tc.tile_pool(name="spool", bufs=4))
        tmpool = ctx.enter_context(tc.tile_pool(name="tmpool", bufs=1))
        psumd = ctx.enter_context(tc.tile_pool(name="psumd", bufs=4, space="PSUM"))
        psuma = ctx.enter_context(tc.tile_pool(name="psuma", bufs=2, space="PSUM"))

        # const loads on gpsimd's DGE so they don't serialize ahead of the
        # first x-chunk DMAs on sync's queues
        wct = const.tile([128, K], f8)
        nc.gpsimd.dma_start(out=wct[:, :], in_=wc[:, :])
        cbt = const.tile([128, 1], f32)
        nc.gpsimd.dma_start(out=cbt[:, :], in_=cb[:, :])

        tmt = tmpool.tile([128, npair], f32)
        nc.vector.memset(tmt[:, :], 0.0)

        # Pair gp reduction engine per PAIR_KIND[gp % 5]; an unmatched
        # trailing exp-group first-half falls back to DVE.
        gp = 0
        c0 = 0
        pend = None   # stashed [128, 2, SEG] psum tile awaiting its 2nd pair
        for cw in sizes:
            xtile = xpool.tile([128, chunk], f8, tag="xc")
            nc.sync.dma_start(out=xtile[:, :cw], in_=xt[:, c0 : c0 + cw])
            for p in range(cw // PAIR):
                s0 = p * PAIR
                s1 = s0 + SEG
                kind = PAIR_KIND[gp % 5]
                if kind == 2 and gp + 1 >= npair:
                    kind = 0
                if kind < 2:
                    ps = psumd.tile([128, SEG], f32)
                    o0, o1 = ps[0:64, :], ps[64:128, :]
                else:
                    if kind == 2:
                        pend = (
                            psuma.tile([128, 2, SEG], f32, name="ps2", tag="ps2"),
                            gp,
                        )
                    ps2 = pend[0]
                    o0 = ps2[0:64, kind - 2, :]
                    o1 = ps2[64:128, kind - 2, :]
                nc.tensor.matmul(
                    o0, wct[:, :], xtile[:, s0 : s0 + SEG],
                    start=True, stop=True, tile_position=(0, 0),
                )
                nc.tensor.matmul(
                    o1, wct[:, :], xtile[:, s1 : s1 + SEG],
                    start=True, stop=True, tile_position=(0, 64),
                )
                if kind < 2:
                    nc.vector.tensor_reduce(
                        tmt[:, gp : gp + 1], ps[:, :],
                        axis=mybir.AxisListType.X, op=mybir.AluOpType.max,
                    )
                elif kind == 3:
                    ps2, gp0 = pend
                    scr = spool.tile([128, 2, SEG], f32)
                    nc.scalar.activation(
                        scr[:, :, :], ps2[:, :, :],
                        mybir.ActivationFunctionType.Exp,
                        bias=cbt[:, :], scale=BETA,
                        accum_out=tmt[:, gp0 : gp0 + 1],
                    )
                    pend = None
                gp += 1
            c0 += cw
        assert pend is None, "unmatched exp-group pair"
        nc.sync.dma_start(out=tm[:, :], in_=tmt[:, :])
    nc.finalize()
    return nc


def _host_prep(x, cluster_centers, tok_per_core, ptok, n_cores):
    """Sort tokens by ||x||^2, build per-core fp8 transposed shards + weights."""
    F8 = _np_f8()
    X = x[0]
    x2_64 = (X.astype(np.float64) ** 2).sum(axis=1)
    perm = np.argsort(x2_64, kind="stable")
    x2s = x2_64[perm]                                   # sorted x2 (fp64)
    X8s = X.astype(F8)[perm]                            # sorted fp8 tokens
    XT = np.ascontiguousarray(X8s.T)                    # [128, n] fp8
    wcT = np.ascontiguousarray((2.0 * cluster_centers.astype(np.float32)).astype(F8).T)

    # exp-screen centers over 2xc (subsample max + margin; BETA=2 tolerates
    # ~+-44 of center error before overflow/flush, and inf/nan columns become
    # unconditional rescore candidates anyway)
    Cf = cluster_centers.astype(np.float32)
    sub = X[:: max(1, X.shape[0] // 32768)][:32768].astype(np.float32)
    c_est = (2.0 * (sub @ Cf.T)).max(axis=0) + 7.0      # (K,)
    cb = np.tile(-BETA * c_est, 2).reshape(128, 1).astype(np.float32)

    in_maps = []
    for c in range(n_cores):
        sl = slice(c * tok_per_core, (c + 1) * tok_per_core)
        xtc = np.zeros((128, ptok), F8)                 # pad tokens: x = 0
        xtc[:, :tok_per_core] = XT[:, sl]
        in_maps.append({"xt": xtc, "wc": wcT, "cb": cb})
    return in_maps, c_est, perm, x2s


def _host_select(x, cluster_centers, tms, c_est, perm, x2s, tok_per_core, n_cores):
    """Bracket true per-segment max of S from the xc screen; rescore candidates.

    Column j, half h covers sorted positions j*PAIR + h*SEG + [0, SEG) of each
    core (kind j%5 in {0,1,2}: DVE max), or additionally the same slice of
    pair j+1 (kind 3: 2-pair ScalarE exp-sum; kind 4 columns are unused).
    Device value v for cluster k (partition h*64+k, column j): DVE -> max_t
    2xc (+-EPS8); ACT -> exp-sum whose log/BETA + c_est[k] is in
    [max_t 2xc, max_t 2xc + ln(1024)/BETA] (+-EPS8). True column max of
    S is in [v' - x2max_s - slack - EPS8, v' - x2min_s + EPS8] with v' the
    (converted) screen value.
    """
    X = x[0]
    Cf = cluster_centers.astype(np.float32)
    c2 = (Cf * Cf).sum(axis=1)
    npair = tms[0].shape[1]
    # column kinds: 0 -> DVE, 3 -> ACT 2-pair group, 4 -> unused
    pk = np.array(PAIR_KIND)[np.arange(npair) % len(PAIR_KIND)]
    kind = np.where(pk == 0, 0, np.where(pk == 2, 3, 4))
    kind[(pk == 2) & (np.arange(npair) + 1 >= npair)] = 0  # unmatched -> DVE

    def col_ranges(j, h):
        """Local sorted-position ranges (clipped) covered by column j, half h."""
        if kind[j] == 4:
            return []
        firsts = [j * PAIR + h * SEG]
        if kind[j] == 3:
            firsts.append((j + 1) * PAIR + h * SEG)
        out = []
        for t0 in firsts:
            t1 = min(t0 + SEG, tok_per_core)
            if t0 < tok_per_core:
                out.append((t0, t1))
        return out

    # per-column x2 ranges / validity / pad-freedom, and per-column LSE slack
    x2min = np.full((n_cores, 2, npair), np.inf)
    x2max = np.full((n_cores, 2, npair), -np.inf)
    valid = np.zeros((n_cores, 2, npair), bool)
    full = np.zeros((n_cores, 2, npair), bool)
    slack = np.where(kind == 3, np.log(2.0 * SEG) / BETA, 0.0)
    for c in range(n_cores):
        base = c * tok_per_core
        for j in range(npair):
            for h in range(2):
                rr = col_ranges(j, h)
                if not rr:
                    continue
                lo = min(x2s[base + t0 : base + t1].min() for t0, t1 in rr)
                hi = max(x2s[base + t0 : base + t1].max() for t0, t1 in rr)
                x2min[c, h, j] = lo
                x2max[c, h, j] = hi
                valid[c, h, j] = True
                nfull = (2 if kind[j] == 3 else 1)
                full[c, h, j] = sum(t1 - t0 for t0, t1 in rr) == nfull * SEG

    stack = np.stack(tms)                                # (ncore, 128, npair)
    vals = np.transpose(
        stack.reshape(n_cores, 2, K, npair), (2, 0, 1, 3)
    ).astype(np.float64)                                 # (K, ncore, half, npair)

    act = kind == 3
    indices = np.zeros(K, np.int64)
    for k in range(K):
        vk = vals[k].copy()                              # (ncore, half, npair)
        ok = valid.copy()
        bad = np.zeros_like(ok)
        # convert exp-sum columns to log-domain upper bounds on max 2xc
        v_act = vk[:, :, act]
        nonfin = ~np.isfinite(v_act)
        zero = (v_act == 0) & ~nonfin
        with np.errstate(divide="ignore"):
            conv = np.log(np.maximum(v_act, 1e-300)) / BETA + c_est[k]
        conv[zero] = PAD_NEG
        conv[nonfin] = PAD_NEG
        vk[:, :, act] = conv
        bad[:, :, act] = nonfin
        # zero exp-sum columns: max 2xc provably < c_est - 87.3/BETA; treat
        # as excluded only if that bound cannot reach the capture floor
        zmask = np.zeros_like(ok)
        zmask[:, :, act] = zero

        upper = np.where(ok, vk - x2min + EPS8, PAD_NEG)
        # lower brackets only from pad-free segments: pad tokens contribute a
        # fake 2xc = 0 to the device max, which must never raise the floor
        lower = np.where(
            ok & full & ~bad & (vk > PAD_NEG / 2),
            vk - x2max - slack[None, None, :] - EPS8,
            PAD_NEG,
        )
        floor = lower.max()
        zbound = c_est[k] - 87.3 / BETA - np.where(ok, x2min, np.inf) + EPS8
        cand = (upper >= floor) | (bad & ok) | (zmask & (zbound >= floor))

        toks = []
        for c, h, j in np.argwhere(cand):
            base = c * tok_per_core
            for t0, t1 in col_ranges(j, h):
                toks.append(perm[base + t0 : base + t1])
        tok = np.unique(np.concatenate(toks))
        seg = X[tok].astype(np.float32)
        d2 = (seg * seg).sum(axis=1) + c2[k] - 2.0 * (seg @ Cf[k])
        indices[k] = tok[int(np.argmin(d2))]
    return X[indices][None]                              # (1, K, 128) fp32


def _run(x, cluster_centers, trace=False, trace_kwargs=None):
    from concourse.bass_utils import run_bass_kernel_spmd

    x = np.asarray(x)
    cluster_centers = np.asarray(cluster_centers)
    nc = _build_nc(PTOK, CHUNK)
    in_maps, c_est, perm, x2s = _host_prep(
        x, cluster_centers, TOK_PER_CORE, PTOK, N_CORES
    )
    res = run_bass_kernel_spmd(
        nc, in_maps, list(range(N_CORES)), trace=trace,
        **(trace_kwargs or {}),
    )
    tms = [res.results[c]["tm"] for c in range(N_CORES)]
    out = _host_select(
        x, cluster_centers, tms, c_est, perm, x2s, TOK_PER_CORE, N_CORES
    )
    return out, res


def kernel(x, cluster_centers):
    return _run(x, cluster_centers)[0]


# revision 47
# speedup vs baseline: 1.0312x; 1.0312x over previous
"""Trainium2 Bass kernel for nn_ClusteringLayer: per-cluster nearest-token retrieval.

reference: d2[t,k] = ||x_t||^2 + ||c_k||^2 - 2 x_t.c_k ; indices[k] = argmin_t d2;
output = x[indices]  (shape (1, 64, 128), fp32).

Strategy (8-way token-parallel, memory-regime):
  * Device pass is a SCREEN, not the final answer. The host sorts tokens by
    ||x||^2, casts to fp8e4m3, pre-transposes to feature-major [128, n], and
    shards the sorted order across the 8 cores. Each core computes only the
    cross term xc[k,t] = (2c_k).x_t (one K=128 fp8 matmul per 512-token
    segment; two segments pack into each [128, 512] PSUM tile via
    column-group tile_position) and reduces each segment to one value per
    cluster. Because tokens are x2-sorted, ||x||^2 is nearly constant within
    a segment, so max_t(2xc - x2) is bracketed by the device's max_t(2xc)
    minus the segment's known [x2min, x2max] — no x2 work on device at all.
  * Per-segment-pair reduction alternates engines: DVE reduce-max for
    gp%2==0, ScalarE exp-sum for the rest (activation(Exp, accum_out): a
    log-sum-exp upper bound with slack <= ln(512)/BETA), splitting the
    PSUM-read wall across both otherwise-idle engines.
  * The host turns the per-(cluster, segment) screen values into upper/lower
    brackets of the true per-segment max of S = 2xc - x2, keeps every
    segment whose upper bracket clears the best lower bracket (sound under
    fp8 error EPS8, LSE slack, and x2 spread), rescores those few segments
    exactly in fp32 with the reference formula, and gathers winners from the
    original fp32 x — the final output is exact.
"""

import numpy as np
import ml_dtypes

BF = ml_dtypes.bfloat16

N_TOKENS = 1_000_000
D = 128
K = 64
N_CORES = 8
SEG = 512                 # tokens per matmul / per reduced segment
PAIR = 2 * SEG            # tokens per PSUM tile (two column-group matmuls)
TOK_PER_CORE = N_TOKENS // N_CORES          # 125000
NPAIR = -(-TOK_PER_CORE // PAIR)            # 123
PTOK = NPAIR * PAIR                         # 125952 (952 pad tokens / core)
CHUNK = 8192              # tokens per DMA chunk (multiple of PAIR)
BETA = 2.0                # exp-screen sharpness; LSE slack = ln(512)/BETA ~ 3.1
# reduction-engine schedule, one entry per gp%5: 0 -> DVE pair, 2/3 -> first/
# second half of a 2-pair ScalarE exp-sum group (3 DVE : 2 ACT measured best)
PAIR_KIND = (0, 0, 0, 2, 3)
EPS8 = 12.0               # 2x max observed fp8 screen error (~5.6)
PAD_NEG = -1.0e9


def _np_f8():
    from concourse import mybir

    return mybir.dt.np(mybir.dt.float8e4)


def _build_nc(ptok, chunk):
    from contextlib import ExitStack

    import concourse.bacc as bacc
    import concourse.tile as tile
    from concourse import mybir

    f32 = mybir.dt.float32
    f8 = mybir.dt.float8e4

    npair = ptok // PAIR
    nc = bacc.Bacc()
    xt = nc.declare_dram_parameter("xt", [128, ptok], f8, isOutput=False)
    wc = nc.declare_dram_parameter("wc", [128, K], f8, isOutput=False)
    cb = nc.declare_dram_parameter("cb", [128, 1], f32, isOutput=False)
    tm = nc.declare_dram_parameter("tm", [128, npair], f32, isOutput=True)

    # ramp-up chunk schedule: small first chunks so the first matmuls start
    # as soon as possible instead of waiting on a full-size DMA
    sizes = []
    rem = ptok
    for s in (1024, 2048, 4096):
        if rem >= s:
            sizes.append(s)
            rem -= s
    while rem > 0:
        s = min(chunk, rem)
        sizes.append(s)
        rem -= s

    with tile.TileContext(nc) as tc, ExitStack() as ctx:
        const = ctx.enter_context(tc.tile_pool(name="const", bufs=1))
        xpool = ctx.enter_context(tc.tile_pool(name="xpool", bufs=4))
        spool = ctx.enter_context(tc.tile_pool(name="spool", bufs=4))
        tmpool = ctx.enter_context(tc.tile_pool(name="tmpool", bufs=1))
        psumd = ctx.enter_context(tc.tile_pool(name="psumd", bufs=4, space="PSUM"))
        psuma = ctx.enter_context(tc.tile_pool(name="psuma", bufs=2, space="PSUM"))

        # const loads on gpsimd's DGE so they don't serialize ahead of the
        # first x-chunk DMAs on sync's queues
        wct = const.tile([128, K], f8)
        nc.gpsimd.dma_start(out=wct[:, :], in_=wc[:, :])
        cbt = const.tile([128, 1], f32)
        nc.gpsimd.dma_start(out=cbt[:, :], in_=cb[:, :])

        tmt = tmpool.tile([128, npair], f32)
        nc.vector.memset(tmt[:, :], 0.0)

        # Pair gp reduction engine per PAIR_KIND[gp % 5]; an unmatched
        # trailing exp-group first-half falls back to DVE.
        gp = 0
        c0 = 0
        pend = None   # stashed [128, 2, SEG] psum tile awaiting its 2nd pair
        for cw in sizes:
            xtile = xpool.tile([128, chunk], f8, tag="xc")
            nc.sync.dma_start(out=xtile[:, :cw], in_=xt[:, c0 : c0 + cw])
            for p in range(cw // PAIR):
                s0 = p * PAIR
                s1 = s0 + SEG
                kind = PAIR_KIND[gp % 5]
                if kind == 2 and gp + 1 >= npair:
                    kind = 0
                if kind < 2:
                    ps = psumd.tile([128, SEG], f32)
                    o0, o1 = ps[0:64, :], ps[64:128, :]
                else:
                    if kind == 2:
                        pend = (
                            psuma.tile([128, 2, SEG], f32, name="ps2", tag="ps2"),
                            gp,
                        )
                    ps2 = pend[0]
                    o0 = ps2[0:64, kind - 2, :]
                    o1 = ps2[64:128, kind - 2, :]
                nc.tensor.matmul(
                    o0, wct[:, :], xtile[:, s0 : s0 + SEG],
                    start=True, stop=True, tile_position=(0, 0),
                )
                nc.tensor.matmul(
                    o1, wct[:, :], xtile[:, s1 : s1 + SEG],
                    start=True, stop=True, tile_position=(0, 64),
                )
                if kind < 2:
                    nc.vector.tensor_reduce(
                        tmt[:, gp : gp + 1], ps[:, :],
                        axis=mybir.AxisListType.X, op=mybir.AluOpType.max,
                    )
                elif kind == 3:
                    ps2, gp0 = pend
                    scr = spool.tile([128, 2, SEG], f32)
                    nc.scalar.activation(
                        scr[:, :, :], ps2[:, :, :],
                        mybir.ActivationFunctionType.Exp,
                        bias=cbt[:, :], scale=BETA,
                        accum_out=tmt[:, gp0 : gp0 + 1],
                    )
                    pend = None
                gp += 1
            c0 += cw
        assert pend is None, "unmatched exp-group pair"
        nc.sync.dma_start(out=tm[:, :], in_=tmt[:, :])
    nc.finalize()
    return nc


def _host_prep(x, cluster_centers, tok_per_core, ptok, n_cores):
    """Sort tokens by ||x||^2, build per-core fp8 transposed shards + weights."""
    F8 = _np_f8()
    X = x[0]
    x2_64 = (X.astype(np.float64) ** 2).sum(axis=1)
    perm = np.argsort(x2_64, kind="stable")
    x2s = x2_64[perm]                                   # sorted x2 (fp64)
    X8s = X.astype(F8)[perm]                            # sorted fp8 tokens
    XT = np.ascontiguousarray(X8s.T)                    # [128, n] fp8
    wcT = np.ascontiguousarray((2.0 * cluster_centers.astype(np.float32)).astype(F8).T)

    # exp-screen centers over 2xc (subsample max + margin; BETA=2 tolerates
    # ~+-44 of center error before overflow/flush, and inf/nan columns become
    # unconditional rescore candidates anyway)
    Cf = cluster_centers.astype(np.float32)
    sub = X[:: max(1, X.shape[0] // 32768)][:32768].astype(np.float32)
    c_est = (2.0 * (sub @ Cf.T)).max(axis=0) + 7.0      # (K,)
    cb = np.tile(-BETA * c_est, 2).reshape(128, 1).astype(np.float32)

    in_maps = []
    for c in range(n_cores):
        sl = slice(c * tok_per_core, (c + 1) * tok_per_core)
        xtc = np.zeros((128, ptok), F8)                 # pad tokens: x = 0
        xtc[:, :tok_per_core] = XT[:, sl]
        in_maps.append({"xt": xtc, "wc": wcT, "cb": cb})
    return in_maps, c_est, perm, x2s


def _host_select(x, cluster_centers, tms, c_est, perm, x2s, tok_per_core, n_cores):
    """Bracket true per-segment max of S from the xc screen; rescore candidates.

    Column j, half h covers sorted positions j*PAIR + h*SEG + [0, SEG) of each
    core (kind j%5 in {0,1,2}: DVE max), or additionally the same slice of
    pair j+1 (kind 3: 2-pair ScalarE exp-sum; kind 4 columns are unused).
    Device value v for cluster k (partition h*64+k, column j): DVE -> max_t
    2xc (+-EPS8); ACT -> exp-sum whose log/BETA + c_est[k] is in
    [max_t 2xc, max_t 2xc + ln(1024)/BETA] (+-EPS8). True column max of
    S is in [v' - x2max_s - slack - EPS8, v' - x2min_s + EPS8] with v' the
    (converted) screen value.
    """
    X = x[0]
    Cf = cluster_centers.astype(np.float32)
    c2 = (Cf * Cf).sum(axis=1)
    npair = tms[0].shape[1]
    # column kinds: 0 -> DVE, 3 -> ACT 2-pair group, 4 -> unused
    pk = np.array(PAIR_KIND)[np.arange(npair) % len(PAIR_KIND)]
    kind = np.where(pk == 0, 0, np.where(pk == 2, 3, 4))
    kind[(pk == 2) & (np.arange(npair) + 1 >= npair)] = 0  # unmatched -> DVE

    def col_ranges(j, h):
        """Local sorted-position ranges (clipped) covered by column j, half h."""
        if kind[j] == 4:
            return []
        firsts = [j * PAIR + h * SEG]
        if kind[j] == 3:
            firsts.append((j + 1) * PAIR + h * SEG)
        out = []
        for t0 in firsts:
            t1 = min(t0 + SEG, tok_per_core)
            if t0 < tok_per_core:
                out.append((t0, t1))
        return out

    # per-column x2 ranges / validity / pad-freedom, and per-column LSE slack
    x2min = np.full((n_cores, 2, npair), np.inf)
    x2max = np.full((n_cores, 2, npair), -np.inf)
    valid = np.zeros((n_cores, 2, npair), bool)
    full = np.zeros((n_cores, 2, npair), bool)
    slack = np.where(kind == 3, np.log(2.0 * SEG) / BETA, 0.0)
    for c in range(n_cores):
        base = c * tok_per_core
        for j in range(npair):
            for h in range(2):
                rr = col_ranges(j, h)
                if not rr:
                    continue
                lo = min(x2s[base + t0 : base + t1].min() for t0, t1 in rr)
                hi = max(x2s[base + t0 : base + t1].max() for t0, t1 in rr)
                x2min[c, h, j] = lo
                x2max[c, h, j] = hi
                valid[c, h, j] = True
                nfull = (2 if kind[j] == 3 else 1)
                full[c, h, j] = sum(t1 - t0 for t0, t1 in rr) == nfull * SEG

    stack = np.stack(tms)                                # (ncore, 128, npair)
    vals = np.transpose(
        stack.reshape(n_cores, 2, K, npair), (2, 0, 1, 3)
    ).astype(np.float64)                                 # (K, ncore, half, npair)

    act = kind == 3
    indices = np.zeros(K, np.int64)
    for k in range(K):
        vk = vals[k].copy()                              # (ncore, half, npair)
        ok = valid.copy()
        bad = np.zeros_like(ok)
        # convert exp-sum columns to log-domain upper bounds on max 2xc
        v_act = vk[:, :, act]
        nonfin = ~np.isfinite(v_act)
        zero = (v_act == 0) & ~nonfin
        with np.errstate(divide="ignore"):
            conv = np.log(np.maximum(v_act, 1e-300)) / BETA + c_est[k]
        conv[zero] = PAD_NEG
        conv[nonfin] = PAD_NEG
        vk[:, :, act] = conv
        bad[:, :, act] = nonfin
        # zero exp-sum columns: max 2xc provably < c_est - 87.3/BETA; treat
        # as excluded only if that bound cannot reach the capture floor
        zmask = np.zeros_like(ok)
        zmask[:, :, act] = zero

        upper = np.where(ok, vk - x2min + EPS8, PAD_NEG)
        # lower brackets only from pad-free segments: pad tokens contribute a
        # fake 2xc = 0 to the device max, which must never raise the floor
        lower = np.where(
            ok & full & ~bad & (vk > PAD_NEG / 2),
            vk - x2max - slack[None, None, :] - EPS8,
            PAD_NEG,
        )
        floor = lower.max()
        zbound = c_est[k] - 87.3 / BETA - np.where(ok, x2min, np.inf) + EPS8
        cand = (upper >= floor) | (bad & ok) | (zmask & (zbound >= floor))

        toks = []
        for c, h, j in np.argwhere(cand):
            base = c * tok_per_core
            for t0, t1 in col_ranges(j, h):
                toks.append(perm[base + t0 : base + t1])
        tok = np.unique(np.concatenate(toks))
        seg = X[tok].astype(np.float32)
        d2 = (seg * seg).sum(axis=1) + c2[k] - 2.0 * (seg @ Cf[k])
        indices[k] = tok[int(np.argmin(d2))]
    return X[indices][None]                              # (1, K, 128) fp32


def _run(x, cluster_centers, trace=False, trace_kwargs=None):
    from concourse.bass_utils import run_bass_kernel_spmd

    x = np.asarray(x)
    cluster_centers = np.asarray(cluster_centers)
    nc = _build_nc(PTOK, CHUNK)
    in_maps, c_est, perm, x2s = _host_prep(
        x, cluster_centers, TOK_PER_CORE, PTOK, N_CORES
    )
    res = run_bass_kernel_spmd(
        nc, in_maps, list(range(N_CORES)), trace=trace,
        **(trace_kwargs or {}),
    )
    tms = [res.results[c]["tm"] for c in range(N_CORES)]
    out = _host_select(
        x, cluster_centers, tms, c_est, perm, x2s, TOK_PER_CORE, N_CORES
    )
    return out, res


def kernel(x, cluster_centers):
    return _run(x, cluster_centers)[0]
